# revision 60
# baseline (speedup 1.0000x reference)
"""Causal multi-head self-attention on 8 Trainium2 NeuronCores.

Sharding: head-parallel. Each of the 8 cores owns 2 of the 16 heads:
it computes Q/K/V for its heads (full sequence), runs causal flash
attention for them entirely on-chip, applies its slice of the output
projection, and writes a full-shape partial output. The host sums the
8 partials.

Layout strategy (no on-device transposes):
  - x is cast to bf16 on host; x^T tiles (d on partitions) are loaded
    via DMA.
  - Q^T, K^T are produced as (128 = [h0|h1] x 64) x t, which is exactly
    the layout the score matmuls need (lhsT = K^T block, rhs = Q^T).
    K^T is pre-scaled by C1EXP/128 at PSUM eviction so scores arrive as
    z = logit*C1EXP/16 ready for both exp paths.
  - Scores are computed transposed, S^T = (k x q), two heads row-packed
    into the two halves of the PE array, landing in adjacent PSUM banks.
  - exp of each k-block is split per head across two engines — Scalar
    table exp (scale fused) and a custom DVE op EXP16_ANT computing
    ((z+c0) + z^2*c1)^16 ~ exp(16z) — halving the latency to a ready
    pt tile; which engine gets which head alternates per k-block so
    neither systematically gates the AV matmuls.
  - V is computed directly in (k x dh) layout (x^T block stationary)
    with a 65th all-ones column, so the AV matmul accumulates both the
    attention output and the softmax denominator (row 64) in one pass.
  - Causal masking: full-width blocks everywhere; after exp, a
    precomputed 0/1 triangle multiply (DVE) zeroes the k>q corner of
    diagonal blocks. Zeros flow through AV and the denominator.

Scheduling (the PE is the bottleneck; everything serves its pacing):
  - ~5us of dummy matmuls at kernel start warm the HAM clock gate to
    8/8 (2.4 GHz) while the first input DMAs land.
  - AV matmuls are software-pipelined three k-blocks behind their
    scores, so exp latency never stalls the in-order PE queue.
  - ctx^T is evicted in 128-column chunks as soon as each diagonal AV
    finalizes those columns (h0 on Scalar, h1 on Vector), instead of a
    2.7us single-engine burst at the q-tile end that stalled the next
    tile and re-throttled the HAM clock gate.
  - 1/denominator rows are partition-broadcast on the PE (tiny
    ones x rec matmuls into PSUM); one DVE multiply normalizes ctx^T.
  - The previous q-tile's output projection and the V transposes are
    emitted as paced fillers inside the attention k-loop, giving the
    PE independent work wherever a dependency bubble could appear.
  - The kernel's last q-tile runs the entire per-chunk output chain
    (den row -> reciprocal -> broadcast -> normalize -> outproj -> DMA)
    inline after each diagonal AV, eliminating the serial tail.
"""

import re
import sys

for _p in ("/opt/trn_rl_repo", "/root/.axon_site/_ro/trn_rl_repo"):
    if _p not in sys.path:
        sys.path.append(_p)

import ml_dtypes
import numpy as np

B = 2
S = 4096
D = 1024
H = 16
DH = 64
N_CORES = 8
HEADS_PER_CORE = H // N_CORES  # 2

# --- custom DVE exp: ((z + c0) + z^2*c1)^16 ~ exp(16 z) ------------------
# Minimax fit of (c0 + c1*w + a*w^2)^16 to exp(16w) (up to a constant
# factor, which softmax cancels) on w = logit/16 in [-0.26, 0.26]:
# +-1.2e-2 max rel. Scores reach the op pre-scaled: z = c1*logit/16.
C1EXP = 1.0092404361624103
EXP_S0 = 1.0009733750068848       # c0
EXP_S1 = 0.4892225470096522       # a/c1^2
KPRESCALE = C1EXP / 128.0         # folded into the K^T eviction (logit=s/8)
ACT_SCALE = 16.0 / C1EXP          # Scalar-engine exp scale on z

_exp16_op = None


def _get_exp16_op():
    global _exp16_op
    if _exp16_op is not None:
        return _exp16_op
    from concourse.dve_spec import C0, C1, Spec, Src0
    import concourse.dve_ops as dops
    from concourse.dve_ops import DveOp

    _z = Src0
    _Q = (_z + C0) + (_z * _z) * C1
    _Q2 = _Q * _Q
    _Q4 = _Q2 * _Q2
    _Q8 = _Q4 * _Q4

    def _ref(in0, in1, s0, s1, imm2):
        z = in0.astype(np.float32)
        q = (z + np.float32(s0)) + z * z * np.float32(s1)
        return (q ** 16).astype(np.float32)

    op = DveOp("EXP16_ANT", Spec(body=_Q8 * _Q8, reference=_ref),
               subdim=False, uops_sha={})
    dops.OPS.append(op)
    dops.CUSTOM_DVE_SPECS[op.name] = op.spec
    dops._SUB_OPCODE_FOR_NAME[op.name] = (
        dops._CUSTOM_DVE_ROW_BASE + len(dops.OPS) - 1)
    try:
        op.compile("v3")
    except ValueError as e:  # harvest the pinned sha from the drift message
        m = re.search(r'="([0-9a-f]+)"', str(e))
        if not (m and "drifted" in str(e)):
            raise
        op.uops_sha["v3"] = m.group(1)
    op.compile("v3")
    _exp16_op = op
    return op


_cache = {}


def _build(nc, b, s):
    import concourse.mybir as mybir
    from concourse.tile import TileContext
    from contextlib import ExitStack

    dt = mybir.dt
    AF = mybir.ActivationFunctionType
    ALU = mybir.AluOpType
    exp16 = _get_exp16_op()

    t_total = b * s          # 8192
    TT = 512                 # t tile (QKV free dim)
    n_ttiles = t_total // TT
    n_dblk = D // 128        # 8
    QT = 512                 # q tile
    n_qt = s // QT           # per batch
    KB = 128                 # k block

    x_d = nc.dram_tensor("xT", [D, t_total], dt.bfloat16, kind="ExternalInput")
    wqkv_d = nc.dram_tensor("wqkvT", [n_dblk, 128, 3 * 128], dt.bfloat16,
                            kind="ExternalInput")
    wout_d = nc.dram_tensor("woutT", [128, D], dt.bfloat16, kind="ExternalInput")
    out_d = nc.dram_tensor("partial_out", [t_total, D], dt.bfloat16,
                           kind="ExternalOutput")

    with TileContext(nc) as tc, ExitStack() as ctx:
        const = ctx.enter_context(tc.tile_pool(name="const", bufs=1))
        wqkvT = const.tile([128, n_dblk, 3 * 128], dt.bfloat16, tag="wqkv")
        woutT = const.tile([128, D], dt.bfloat16, tag="wout")
        qT = const.tile([128, t_total], dt.bfloat16, tag="qT")
        kT = const.tile([128, t_total], dt.bfloat16, tag="kT")
        n_kblk = s // KB  # 32
        v65 = const.tile([128, b, HEADS_PER_CORE, n_kblk, DH + 1], dt.bfloat16,
                         tag="v65")
        ctxT = const.tile([128, t_total], dt.bfloat16, tag="ctxT")
        tri = const.tile([128, 128], dt.bfloat16, tag="tri")
        ident = const.tile([128, 128], dt.bfloat16, tag="ident")

        # weights on the Activation HWDGE queue: they land in parallel with
        # the startup x strips on the SP queue (serial before: the first QKV
        # matmul could not start until ~12.4us; now ~9.3us)
        nc.scalar.dma_start(wqkvT[:], wqkv_d.rearrange("k p e -> p k e"))
        nc.scalar.dma_start(woutT[:], wout_d[:])

        # HAM warmup: ~5us of dummy matmuls on a zeroed tile while the
        # first input DMAs land, so the PE clock gate is at 8/8 (2.4 GHz)
        # when real work starts instead of ramping mid-QKV.
        warm = const.tile([128, 512], dt.bfloat16, tag="warm")
        nc.vector.memset(warm[:], 0.0)
        ones1 = const.tile([1, DH], dt.bfloat16, tag="ones1")
        nc.vector.memset(ones1[:], 1.0)

        nc.gpsimd.memset(v65[:, :, :, :, DH], 1.0)
        nc.gpsimd.memset(tri[:], 1.0)
        nc.gpsimd.affine_select(
            tri[:], tri[:], pattern=[[1, 128]], compare_op=ALU.is_ge,
            fill=0.0, base=0, channel_multiplier=-1,
        )
        nc.gpsimd.affine_select(
            ident[:], tri[:], pattern=[[1, 128]], compare_op=ALU.is_equal,
            fill=0.0, base=0, channel_multiplier=-1,
        )

        xt_pool = ctx.enter_context(tc.tile_pool(name="xt", bufs=32))
        # PSUM budget 8 banks: scores 2-bank tiles x2 bufs = 4, two 1-bank
        # AV accumulators = 2, one 2-buf 1-bank pool shared by the QKV and
        # output projections = 2.
        sc_ps = ctx.enter_context(tc.tile_pool(name="sc_ps", bufs=2, space="PSUM"))
        o65_ps = ctx.enter_context(tc.tile_pool(name="o65_ps", bufs=1, space="PSUM"))
        pj_ps = ctx.enter_context(tc.tile_pool(name="pj_ps", bufs=2, space="PSUM"))
        pt_pool = ctx.enter_context(tc.tile_pool(name="pt", bufs=8))
        vt_pool = ctx.enter_context(tc.tile_pool(name="vt", bufs=2))
        ev_pool = ctx.enter_context(tc.tile_pool(name="ev", bufs=6))
        out_sb_pool = ctx.enter_context(tc.tile_pool(name="out_sb", bufs=6))
        rec_saved = {}

        for _w in range(12):
            ps_w = pj_ps.tile([128, 512], dt.float32, tag="pj")
            nc.tensor.matmul(ps_w[:], warm[:, 0:128], warm[:])

        xt_pref = {}

        def prefetch_xt(tt, n_tt=1):
            """Issue the x^T loads for n_tt consecutive t-tiles one iteration
            early, so they sit AHEAD of the output-store DMAs in the SP queue
            (whose deps resolve mid-iteration and would head-of-line-block
            these). Consecutive tiles share one wide DMA per d-block."""
            t0 = tt * TT
            tiles = []
            for dd in range(n_dblk):
                xt = xt_pool.tile([128, n_tt * TT], dt.bfloat16, tag="xt")
                nc.sync.dma_start(
                    xt[:], x_d[dd * 128:(dd + 1) * 128, t0:t0 + n_tt * TT])
                tiles.append(xt)
            for k in range(n_tt):
                xt_pref[tt + k] = [
                    t[:, k * TT:(k + 1) * TT] for t in tiles]

        def qkv_ttile(tt):
            """QKV projection for t-range [tt*TT, (tt+1)*TT)."""
            t0 = tt * TT
            ps_q = pj_ps.tile([128, TT], dt.float32, tag="pj")
            ps_k = pj_ps.tile([128, TT], dt.float32, tag="pj")
            xts = xt_pref.pop(tt)
            for dd in range(n_dblk):
                xt = xts[dd]
                st = dict(start=(dd == 0), stop=(dd == n_dblk - 1))
                nc.tensor.matmul(ps_q[:], wqkvT[:, dd, 0:128], xt, **st)
                nc.tensor.matmul(ps_k[:], wqkvT[:, dd, 128:256], xt, **st)
            # casts on the Scalar engine: its slack is in the QKV phase, and
            # keeping the DVE queue exp-only lets the PE's AV stream run
            # without waiting behind bulk copies.
            nc.scalar.copy(qT[:, t0:t0 + TT], ps_q[:])
            # K^T pre-scaled so scores arrive as z = logit*C1EXP/16
            nc.scalar.mul(kT[:, t0:t0 + TT], ps_k[:], KPRESCALE)
            ps_vt = pj_ps.tile([128, TT], dt.float32, tag="pj")
            for dd in range(n_dblk):
                st = dict(start=(dd == 0), stop=(dd == n_dblk - 1))
                nc.tensor.matmul(ps_vt[:], wqkvT[:, dd, 256:384],
                                 xts[dd], **st)
            vt = vt_pool.tile([128, TT], dt.bfloat16, tag="vt")
            # on Vector: splits the t-tile-boundary eviction burst across
            # engines (Scalar otherwise backs up ~2.4us right here)
            nc.vector.tensor_copy(vt[:], ps_vt[:])

            def finish_v():
                # V^T (e x t) -> V (t x e) via PE transpose, 128x128 blocks.
                # Deferred into the attention loop (filler) so the PE does
                # not stall on the vt cast at the t-tile boundary.
                ps_tv = pj_ps.tile([128, TT], dt.bfloat16, tag="pj")
                for j in range(TT // 128):
                    nc.tensor.transpose(ps_tv[:, j * 128:(j + 1) * 128],
                                        vt[:, j * 128:(j + 1) * 128],
                                        ident[:])
                bb = t0 // s
                for j in range(TT // 128):
                    kb = (t0 % s) // KB + j
                    # both heads in one strided copy (dst [128, 2, 64])
                    nc.scalar.copy(
                        v65[:, bb, :, kb, 0:DH],
                        ps_tv[:, j * 128:(j + 1) * 128].rearrange(
                            "p (h e) -> p h e", h=HEADS_PER_CORE))

            return finish_v

        def attention(bb, qt, fillers=(), final=False):
            """One q-tile of causal attention for both heads of batch bb.
            `fillers`: closures emitting independent PE work (outproj blocks,
            V transposes), paced one per two k-blocks to absorb exp latency
            and keep the PE warm. `final`: this is the kernel's last q-tile —
            pipeline the whole output chain per 128-column diagonal chunk
            instead of leaving a serial outproj tail."""
            fillers = list(fillers)
            nkb_ = (qt + 1) * QT // KB
            f_stride = max(1, (nkb_ - 2) // max(1, len(fillers)))
            tq0 = bb * s + qt * QT
            o65_h0 = o65_ps.tile([DH + 1, QT], dt.float32, tag="o65h0")
            o65_h1 = o65_ps.tile([DH + 1, QT], dt.float32, tag="o65h1")
            nkb = (qt + 1) * QT // KB
            if final:
                frow = [ev_pool.tile([1, QT], dt.float32, tag="row",
                                     name=f"frow{h}") for h in range(2)]
                frec = [ev_pool.tile([1, QT], dt.float32, tag="rec",
                                     name=f"frec{h}") for h in range(2)]
                frecb = [ev_pool.tile([1, QT], dt.bfloat16, tag="recb",
                                      name=f"frecb{h}") for h in range(2)]

            def emit_av(pt, qc0, w, kb):
                st = dict(start=(kb == 0), stop=(kb == nkb - 1))
                nc.tensor.matmul(o65_h0[:, qc0:QT], v65[:, bb, 0, kb, :],
                                 pt[:, 0:w], **st)
                nc.tensor.matmul(o65_h1[:, qc0:QT], v65[:, bb, 1, kb, :],
                                 pt[:, QT:QT + w], **st)
                j = kb - qt * (QT // KB)
                if j >= 0:
                    # after diagonal AV j, ctx columns [j*KB,(j+1)*KB) are
                    # final: evict them now, split across Scalar/Vector.
                    # Chunking kills the 2.7us scalar burst at the tile end
                    # (which stalled the next tile's matmuls and tripped the
                    # HAM clock gate back to 1.2 GHz).
                    c0, c1 = j * KB, (j + 1) * KB
                    nc.scalar.copy(ctxT[0:DH, tq0 + c0:tq0 + c1],
                                   o65_h0[0:DH, c0:c1])
                    nc.vector.tensor_copy(ctxT[DH:2 * DH, tq0 + c0:tq0 + c1],
                                          o65_h1[0:DH, c0:c1])
                    if final:
                        # this 128-col chunk is complete: run its whole
                        # output chain now so no serial tail remains.
                        nc.scalar.copy(frow[0][:, c0:c1],
                                       o65_h0[DH:DH + 1, c0:c1])
                        nc.vector.tensor_copy(frow[1][:, c0:c1],
                                              o65_h1[DH:DH + 1, c0:c1])
                        for h in range(2):
                            nc.vector.reciprocal_approx_fast(
                                frec[h][:, c0:c1], frow[h][:, c0:c1])
                        nc.scalar.copy(frecb[0][:, c0:c1],
                                       frec[0][:, c0:c1])
                        nc.vector.tensor_copy(frecb[1][:, c0:c1],
                                              frec[1][:, c0:c1])
                        fbc = pj_ps.tile([128, KB], dt.float32, tag="pj")
                        nc.tensor.matmul(fbc[0:DH, :], ones1[:, 0:DH],
                                         frecb[0][:, c0:c1],
                                         tile_position=(0, 0))
                        nc.tensor.matmul(fbc[DH:2 * DH, :], ones1[:, 0:DH],
                                         frecb[1][:, c0:c1],
                                         tile_position=(0, 64))
                        sl = ctxT[:, tq0 + c0:tq0 + c1]
                        nc.vector.tensor_tensor(sl, sl, fbc[:], ALU.mult)
                        t0 = tq0 + c0
                        ob = out_sb_pool.tile([128, D], dt.bfloat16,
                                              tag="ob")
                        for e in range(D // 512):
                            ps = pj_ps.tile([128, 512], dt.float32,
                                            tag="pj")
                            nc.tensor.matmul(ps[:], ctxT[:, t0:t0 + 128],
                                             woutT[:, e * 512:(e + 1) * 512])
                            dst = ob[:, e * 512:(e + 1) * 512]
                            if e == 0:
                                nc.scalar.copy(dst, ps[:])
                            else:
                                nc.vector.tensor_copy(dst, ps[:])
                        nc.sync.dma_start(out_d[t0:t0 + 128, :], ob[:])

            pending = []  # AV lagged three k-blocks so exp latency is hidden
            for kb in range(nkb):
                tk0 = bb * s + kb * KB
                j = kb - qt * (QT // KB)  # >= 0 on the diagonal
                qc0 = max(j, 0) * KB      # first valid local q column
                w = QT - qc0
                ps_s = sc_ps.tile([128, 2 * QT], dt.float32, tag="ps_s")
                nc.tensor.matmul(ps_s[:, 0:w], kT[0:64, tk0:tk0 + KB],
                                 qT[0:64, tq0 + qc0:tq0 + QT],
                                 tile_position=(0, 0))
                nc.tensor.matmul(ps_s[:, QT:QT + w], kT[64:128, tk0:tk0 + KB],
                                 qT[64:128, tq0 + qc0:tq0 + QT],
                                 tile_position=(64, 0))
                pt = pt_pool.tile([128, 2 * QT], dt.bfloat16, tag="pt")
                # exp split per head every iteration: Scalar table-exp on one
                # half, DVE EXP16_ANT on the other — halves the latency to
                # pt readiness vs one 1024-wide op, and loads both engines
                # evenly (each ~0.6us per 790ns of PE work). Alternate which
                # engine gets which head so neither systematically gates the
                # first AV matmul of the (lagged) pair.
                sc_half = (kb % 2) * QT
                ve_half = QT - sc_half
                nc.scalar.activation(pt[:, sc_half:sc_half + w],
                                     ps_s[:, sc_half:sc_half + w], AF.Exp,
                                     scale=ACT_SCALE)
                nc.vector._custom_dve(
                    exp16, out=pt[:, ve_half:ve_half + w],
                    in0=ps_s[:, ve_half:ve_half + w],
                    s0=EXP_S0, s1=EXP_S1)
                if j >= 0:
                    for half in (QT, 0):
                        # zero the k>q corner (first KB valid columns)
                        nc.vector.tensor_tensor(
                            pt[:, half:half + KB], pt[:, half:half + KB],
                            tri[:], ALU.mult)
                pending.append((pt, qc0, w, kb))
                if len(pending) > 3:
                    emit_av(*pending.pop(0))
                if fillers and kb >= 2 and (kb - 2) % f_stride == 0:
                    fillers.pop(0)()
            for p in pending:
                emit_av(*p)
            for f in fillers:
                f()
            if final:
                return
            # ctx itself was evicted in chunks along the diagonal; only the
            # denominator rows remain. One head per engine, in parallel.
            for h, o65 in ((0, o65_h0), (1, o65_h1)):
                row = ev_pool.tile([1, QT], dt.float32, tag="row")
                rec = ev_pool.tile([1, QT], dt.float32, tag="rec")
                recb = ev_pool.tile([1, QT], dt.bfloat16, tag="recb")
                if h == 0:
                    nc.scalar.copy(row[:], o65[DH:DH + 1, :])
                else:
                    nc.vector.tensor_copy(row[:], o65[DH:DH + 1, :])
                nc.vector.reciprocal_approx_fast(rec[:], row[:])
                # bf16 now (tiny op) so the norm filler's broadcast matmul
                # has a ready all-bf16 operand and never stalls the PE
                if h == 0:
                    nc.scalar.copy(recb[:], rec[:])
                else:
                    nc.vector.tensor_copy(recb[:], rec[:])
                rec_saved[(bb, qt, h)] = recb

        def outproj_parts(bb, qt):
            """Output projection for q-tile (bb, qt) as a list of closures
            (norm + 4 t-blocks), to be emitted as attention fillers."""
            tq0 = bb * s + qt * QT

            def norm():
                # partition-broadcast the reciprocals on the PE (ones x rec
                # into PSUM, one 64-col tile per head): ~0.4us of PE work,
                # then a single DVE multiply (SB x PSUM, both base-0)
                # normalizes ctx^T in place.
                rec0 = rec_saved.pop((bb, qt, 0))
                rec1 = rec_saved.pop((bb, qt, 1))
                bc_ps = pj_ps.tile([128, QT], dt.float32, tag="pj")
                nc.tensor.matmul(bc_ps[0:DH, :], ones1[:, 0:DH], rec0[:],
                                 tile_position=(0, 0))
                nc.tensor.matmul(bc_ps[DH:2 * DH, :], ones1[:, 0:DH],
                                 rec1[:], tile_position=(0, 64))
                sl = ctxT[:, tq0:tq0 + QT]
                nc.vector.tensor_tensor(sl, sl, bc_ps[:], ALU.mult)

            def tb(tb4):
                t0 = bb * s + qt * QT + tb4 * 128
                ob = out_sb_pool.tile([128, D], dt.bfloat16, tag="ob")
                for e in range(D // 512):
                    ps = pj_ps.tile([128, 512], dt.float32, tag="pj")
                    nc.tensor.matmul(ps[:], ctxT[:, t0:t0 + 128],
                                     woutT[:, e * 512:(e + 1) * 512])
                    dst = ob[:, e * 512:(e + 1) * 512]
                    if (2 * tb4 + e) % 2 == 0:
                        nc.scalar.copy(dst, ps[:])
                    else:
                        nc.vector.tensor_copy(dst, ps[:])
                nc.sync.dma_start(out_d[t0:t0 + 128, :], ob[:])

            return [norm] + [
                (lambda tb4=tb4: tb(tb4)) for tb4 in range(QT // 128)]

        # Emission: QKV t-tiles interleaved with the attention q-tiles they
        # unblock. The output projection of the PREVIOUS q-tile and the V
        # transposes are spread as fillers inside the attention k-loop, so
        # the in-order PE always has independent work while the exp engines
        # catch up, and the kernel tail shrinks to one q-tile's outproj.
        tiles_per_batch = n_ttiles // b  # == n_qt
        last = tiles_per_batch - 1
        # startup: narrow (1-tile) prefetches, interleaved across batches,
        # so qkv(batch1, t-tile 0) isn't queued behind ~11us of batch-0 DMA
        # (which left the PE idle until ~23us and tripped the HAM gate).
        for k in range(2):
            for bb in range(b):
                prefetch_xt(bb * tiles_per_batch + k, n_tt=1)
        for bb in range(b):
            prefetch_xt(bb * tiles_per_batch + 2, n_tt=2)
        for tt in range(tiles_per_batch):
            qkv_fin = [qkv_ttile(bb * tiles_per_batch + tt)
                       for bb in range(b)]
            for bb in range(b):
                fill = []
                if tt == 0:
                    # q-tile 0's first AV consumes this tile's fresh V
                    # blocks almost immediately: emit the transposes now
                    # (a filler slot would deadlock the in-order PE queue)
                    qkv_fin[bb]()
                else:
                    # norm first: its DVE multiply is the op later score
                    # matmuls queue behind; the earliest filler slot (kb=2)
                    # has the most off-diagonal DVE slack to absorb it
                    op = outproj_parts(bb, tt - 1)
                    fill.append(op[0])
                    fill.append(qkv_fin[bb])
                    fill += op[1:]
                fin = tt == last and bb == b - 1
                if fin:
                    fill += outproj_parts(0, tt)
                attention(bb, tt, fill, final=fin)
            # prefetch AFTER the out-stores: the outs then sit at the SP FIFO
            # head (ob buffers recycle promptly) while the 2-iteration lead
            # still lands x^T well before its QKV consumer
            if tt % 2 == 1 and tt + 3 < tiles_per_batch:
                for bb in range(b):
                    prefetch_xt(bb * tiles_per_batch + tt + 3, n_tt=2)

    return nc


def _get_kernel(b, s):
    key = (b, s)
    if key not in _cache:
        from concourse import bacc
        nc = bacc.Bacc()
        _build(nc, b, s)
        nc.finalize()
        _cache[key] = nc
    return _cache[key]


def _prep_inputs(x, Wqkv, Wout):
    """Host-side shard + transpose + bf16 cast. Returns list of in_maps."""
    b, s, d = x.shape
    xT = np.ascontiguousarray(
        x.reshape(b * s, d).astype(ml_dtypes.bfloat16).T)  # (d, b*s)
    n_dblk = d // 128
    in_maps = []
    for i in range(N_CORES):
        r0 = i * 128
        wq = Wqkv[r0:r0 + 128]            # (128, d)
        wk = Wqkv[d + r0:d + r0 + 128]
        wv = Wqkv[2 * d + r0:2 * d + r0 + 128]
        wT = np.concatenate([wq.T, wk.T, wv.T], axis=1)  # (d, 384)
        wT = wT.reshape(n_dblk, 128, 3 * 128).astype(ml_dtypes.bfloat16)
        woT = Wout[:, r0:r0 + 128].T.astype(ml_dtypes.bfloat16)
        woT = np.ascontiguousarray(woT)
        in_maps.append({"xT": xT, "wqkvT": wT, "woutT": woT})
    return in_maps


_runner_cache = {}


def _make_runner(nc, n_cores):
    """Like bass2jax.run_bass_via_pjrt but with the jitted executable built
    once and cached, and output zero-buffers created on-device."""
    import jax
    from jax.sharding import Mesh, PartitionSpec
    from jax.experimental.shard_map import shard_map
    import concourse.mybir as mybir
    from concourse import bass2jax

    bass2jax.install_neuronx_cc_hook()
    partition_name = (nc.partition_id_tensor.name
                      if nc.partition_id_tensor else None)
    in_names, out_names, out_avals = [], [], []
    for alloc in nc.m.functions[0].allocations:
        if not isinstance(alloc, mybir.MemoryLocationSet):
            continue
        name = alloc.memorylocations[0].name
        if alloc.kind == "ExternalInput":
            if name != partition_name:
                in_names.append(name)
        elif alloc.kind == "ExternalOutput":
            out_names.append(name)
            out_avals.append(jax.core.ShapedArray(
                tuple(alloc.tensor_shape), mybir.dt.np(alloc.dtype)))
    n_params = len(in_names)
    n_outs = len(out_names)
    bind_names = list(in_names) + list(out_names)
    if partition_name is not None:
        bind_names.append(partition_name)

    def _body(*args):
        operands = list(args)
        if partition_name is not None:
            operands.append(bass2jax.partition_id_tensor())
        outs = bass2jax._bass_exec_p.bind(
            *operands,
            out_avals=tuple(out_avals),
            in_names=tuple(bind_names),
            out_names=tuple(out_names),
            lowering_input_output_aliases=(),
            sim_require_finite=True,
            sim_require_nnan=True,
            nc=nc,
        )
        return tuple(outs)

    devices = jax.devices()[:n_cores]
    mesh = Mesh(np.array(devices), ("core",))
    sharded = jax.jit(
        shard_map(
            _body, mesh=mesh,
            in_specs=(PartitionSpec("core"),) * (n_params + n_outs),
            out_specs=(PartitionSpec("core"),) * n_outs,
            check_rep=False),
        donate_argnums=tuple(range(n_params, n_params + n_outs)),
        keep_unused=True)

    def run(in_maps):
        concat_in = [
            np.concatenate([np.asarray(m[name]) for m in in_maps], axis=0)
            for name in in_names]
        concat_zeros = [
            np.zeros((n_cores * a.shape[0], *a.shape[1:]), a.dtype)
            for a in out_avals]
        out_arrs = sharded(*concat_in, *concat_zeros)
        return [
            {name: np.asarray(out_arrs[i]).reshape(
                n_cores, *out_avals[i].shape)[c]
             for i, name in enumerate(out_names)}
            for c in range(n_cores)]

    return run


def kernel(x, Wqkv, Wout, _trace=False):
    b, s, d = x.shape
    nc = _get_kernel(b, s)
    in_maps = _prep_inputs(np.asarray(x), np.asarray(Wqkv), np.asarray(Wout))
    if _trace:
        from concourse.bass_utils import run_bass_kernel_spmd
        res = run_bass_kernel_spmd(nc, in_maps,
                                   core_ids=list(range(N_CORES)), trace=True)
        results = res.results
        kernel.last_results = res
    else:
        key = id(nc)
        if key not in _runner_cache:
            _runner_cache[key] = _make_runner(nc, N_CORES)
        results = _runner_cache[key](in_maps)
    acc = results[0]["partial_out"].astype(np.float32)
    for i in range(1, N_CORES):
        acc = acc + results[i]["partial_out"]
    return acc.reshape(b, s, d)



# revision 65
# speedup vs baseline: 1.0108x; 1.0108x over previous
"""Causal multi-head self-attention on 8 Trainium2 NeuronCores.

Sharding: head-parallel. Each of the 8 cores owns 2 of the 16 heads:
it computes Q/K/V for its heads (full sequence), runs causal flash
attention for them entirely on-chip, applies its slice of the output
projection, and writes a full-shape partial output. The host sums the
8 partials.

Layout strategy (no on-device transposes):
  - x is cast to bf16 on host; x^T tiles (d on partitions) are loaded
    via DMA.
  - Q^T, K^T are produced as (128 = [h0|h1] x 64) x t, which is exactly
    the layout the score matmuls need (lhsT = K^T block, rhs = Q^T).
    K^T is pre-scaled by C1EXP/128 at PSUM eviction so scores arrive as
    z = logit*C1EXP/16 ready for both exp paths.
  - Scores are computed transposed, S^T = (k x q), two heads row-packed
    into the two halves of the PE array, landing in adjacent PSUM banks.
  - exp of each k-block is split per head across two engines — Scalar
    table exp (scale fused) and a custom DVE op EXP16_ANT computing
    ((z+c0) + z^2*c1)^16 ~ exp(16z) — halving the latency to a ready
    pt tile; which engine gets which head alternates per k-block so
    neither systematically gates the AV matmuls.
  - V is computed directly in (k x dh) layout (x^T block stationary)
    with a 65th all-ones column, so the AV matmul accumulates both the
    attention output and the softmax denominator (row 64) in one pass.
  - Causal masking: full-width blocks everywhere; after exp, a
    precomputed 0/1 triangle multiply (DVE) zeroes the k>q corner of
    diagonal blocks. Zeros flow through AV and the denominator.

Scheduling (the PE is the bottleneck; everything serves its pacing):
  - ~5us of dummy matmuls at kernel start warm the HAM clock gate to
    8/8 (2.4 GHz) while the first input DMAs land.
  - AV matmuls are software-pipelined three k-blocks behind their
    scores, so exp latency never stalls the in-order PE queue.
  - ctx^T is evicted in 128-column chunks as soon as each diagonal AV
    finalizes those columns (h0 on Scalar, h1 on Vector), instead of a
    2.7us single-engine burst at the q-tile end that stalled the next
    tile and re-throttled the HAM clock gate.
  - 1/denominator rows are partition-broadcast on the PE (tiny
    ones x rec matmuls into PSUM); one DVE multiply normalizes ctx^T.
  - The previous q-tile's output projection and the V transposes are
    emitted as paced fillers inside the attention k-loop, giving the
    PE independent work wherever a dependency bubble could appear.
  - The kernel's last q-tile runs the entire per-chunk output chain
    (den row -> reciprocal -> broadcast -> normalize -> outproj -> DMA)
    inline after each diagonal AV, eliminating the serial tail.
"""

import re
import sys

for _p in ("/opt/trn_rl_repo", "/root/.axon_site/_ro/trn_rl_repo"):
    if _p not in sys.path:
        sys.path.append(_p)

import ml_dtypes
import numpy as np

B = 2
S = 4096
D = 1024
H = 16
DH = 64
N_CORES = 8
HEADS_PER_CORE = H // N_CORES  # 2

# --- custom DVE exp: ((z + c0) + z^2*c1)^16 ~ exp(16 z) ------------------
# Minimax fit of (c0 + c1*w + a*w^2)^16 to exp(16w) (up to a constant
# factor, which softmax cancels) on w = logit/16 in [-0.26, 0.26]:
# +-1.2e-2 max rel. Scores reach the op pre-scaled: z = c1*logit/16.
C1EXP = 1.0092404361624103
EXP_S0 = 1.0009733750068848       # c0
EXP_S1 = 0.4892225470096522       # a/c1^2
KPRESCALE = C1EXP / 128.0         # folded into the K^T eviction (logit=s/8)
ACT_SCALE = 16.0 / C1EXP          # Scalar-engine exp scale on z

_exp16_op = None


def _get_exp16_op():
    global _exp16_op
    if _exp16_op is not None:
        return _exp16_op
    from concourse.dve_spec import C0, C1, Spec, Src0
    import concourse.dve_ops as dops
    from concourse.dve_ops import DveOp

    _z = Src0
    _Q = (_z + C0) + (_z * _z) * C1
    _Q2 = _Q * _Q
    _Q4 = _Q2 * _Q2
    _Q8 = _Q4 * _Q4

    def _ref(in0, in1, s0, s1, imm2):
        z = in0.astype(np.float32)
        q = (z + np.float32(s0)) + z * z * np.float32(s1)
        return (q ** 16).astype(np.float32)

    op = DveOp("EXP16_ANT", Spec(body=_Q8 * _Q8, reference=_ref),
               subdim=False, uops_sha={})
    dops.OPS.append(op)
    dops.CUSTOM_DVE_SPECS[op.name] = op.spec
    dops._SUB_OPCODE_FOR_NAME[op.name] = (
        dops._CUSTOM_DVE_ROW_BASE + len(dops.OPS) - 1)
    try:
        op.compile("v3")
    except ValueError as e:  # harvest the pinned sha from the drift message
        m = re.search(r'="([0-9a-f]+)"', str(e))
        if not (m and "drifted" in str(e)):
            raise
        op.uops_sha["v3"] = m.group(1)
    op.compile("v3")
    _exp16_op = op
    return op


_cache = {}


def _build(nc, b, s):
    import concourse.mybir as mybir
    from concourse.tile import TileContext
    from contextlib import ExitStack

    dt = mybir.dt
    AF = mybir.ActivationFunctionType
    ALU = mybir.AluOpType
    exp16 = _get_exp16_op()

    t_total = b * s          # 8192
    TT = 512                 # t tile (QKV free dim)
    n_ttiles = t_total // TT
    n_dblk = D // 128        # 8
    QT = 512                 # q tile
    n_qt = s // QT           # per batch
    KB = 128                 # k block

    x_d = nc.dram_tensor("xT", [D, t_total], dt.bfloat16, kind="ExternalInput")
    wqkv_d = nc.dram_tensor("wqkvT", [n_dblk, 128, 3 * 128], dt.bfloat16,
                            kind="ExternalInput")
    wout_d = nc.dram_tensor("woutT", [128, D], dt.bfloat16, kind="ExternalInput")
    out_d = nc.dram_tensor("partial_out", [t_total, D], dt.bfloat16,
                           kind="ExternalOutput")

    with TileContext(nc) as tc, ExitStack() as ctx:
        const = ctx.enter_context(tc.tile_pool(name="const", bufs=1))
        wqkvT = const.tile([128, n_dblk, 3 * 128], dt.bfloat16, tag="wqkv")
        woutT = const.tile([128, D], dt.bfloat16, tag="wout")
        qT = const.tile([128, t_total], dt.bfloat16, tag="qT")
        kT = const.tile([128, t_total], dt.bfloat16, tag="kT")
        n_kblk = s // KB  # 32
        v65 = const.tile([128, b, HEADS_PER_CORE, n_kblk, DH + 1], dt.bfloat16,
                         tag="v65")
        ctxT = const.tile([128, t_total], dt.bfloat16, tag="ctxT")
        tri = const.tile([128, 128], dt.bfloat16, tag="tri")
        ident = const.tile([128, 128], dt.bfloat16, tag="ident")

        # weights on the Activation HWDGE queue: they land in parallel with
        # the startup x strips on the SP queue (serial before: the first QKV
        # matmul could not start until ~12.4us; now ~9.3us)
        nc.scalar.dma_start(wqkvT[:], wqkv_d.rearrange("k p e -> p k e"))
        nc.scalar.dma_start(woutT[:], wout_d[:])

        # HAM warmup: ~5us of dummy matmuls on a zeroed tile while the
        # first input DMAs land, so the PE clock gate is at 8/8 (2.4 GHz)
        # when real work starts instead of ramping mid-QKV.
        warm = const.tile([128, 512], dt.bfloat16, tag="warm")
        nc.vector.memset(warm[:], 0.0)
        ones1 = const.tile([1, DH], dt.bfloat16, tag="ones1")
        nc.vector.memset(ones1[:], 1.0)

        nc.gpsimd.memset(v65[:, :, :, :, DH], 1.0)
        nc.gpsimd.memset(tri[:], 1.0)
        nc.gpsimd.affine_select(
            tri[:], tri[:], pattern=[[1, 128]], compare_op=ALU.is_ge,
            fill=0.0, base=0, channel_multiplier=-1,
        )
        nc.gpsimd.affine_select(
            ident[:], tri[:], pattern=[[1, 128]], compare_op=ALU.is_equal,
            fill=0.0, base=0, channel_multiplier=-1,
        )

        xt_pool = ctx.enter_context(tc.tile_pool(name="xt", bufs=32))
        # PSUM budget 8 banks: scores 2-bank tiles x2 bufs = 4, two 1-bank
        # AV accumulators = 2, one 2-buf 1-bank pool shared by the QKV and
        # output projections = 2.
        sc_ps = ctx.enter_context(tc.tile_pool(name="sc_ps", bufs=2, space="PSUM"))
        o65_ps = ctx.enter_context(tc.tile_pool(name="o65_ps", bufs=1, space="PSUM"))
        pj_ps = ctx.enter_context(tc.tile_pool(name="pj_ps", bufs=2, space="PSUM"))
        pt_pool = ctx.enter_context(tc.tile_pool(name="pt", bufs=8))
        vt_pool = ctx.enter_context(tc.tile_pool(name="vt", bufs=2))
        ev_pool = ctx.enter_context(tc.tile_pool(name="ev", bufs=6))
        out_sb_pool = ctx.enter_context(tc.tile_pool(name="out_sb", bufs=6))
        rec_saved = {}

        for _w in range(12):
            ps_w = pj_ps.tile([128, 512], dt.float32, tag="pj")
            nc.tensor.matmul(ps_w[:], warm[:, 0:128], warm[:])

        xt_pref = {}

        def prefetch_xt(tt, n_tt=1):
            """Issue the x^T loads for n_tt consecutive t-tiles one iteration
            early, so they sit AHEAD of the output-store DMAs in the SP queue
            (whose deps resolve mid-iteration and would head-of-line-block
            these). Consecutive tiles share one wide DMA per d-block."""
            t0 = tt * TT
            tiles = []
            for dd in range(n_dblk):
                xt = xt_pool.tile([128, n_tt * TT], dt.bfloat16, tag="xt")
                nc.sync.dma_start(
                    xt[:], x_d[dd * 128:(dd + 1) * 128, t0:t0 + n_tt * TT])
                tiles.append(xt)
            for k in range(n_tt):
                xt_pref[tt + k] = [
                    t[:, k * TT:(k + 1) * TT] for t in tiles]

        def qkv_ttile(tt):
            """QKV projection for t-range [tt*TT, (tt+1)*TT)."""
            t0 = tt * TT
            ps_q = pj_ps.tile([128, TT], dt.float32, tag="pj")
            ps_k = pj_ps.tile([128, TT], dt.float32, tag="pj")
            xts = xt_pref.pop(tt)
            for dd in range(n_dblk):
                xt = xts[dd]
                st = dict(start=(dd == 0), stop=(dd == n_dblk - 1))
                nc.tensor.matmul(ps_q[:], wqkvT[:, dd, 0:128], xt, **st)
                nc.tensor.matmul(ps_k[:], wqkvT[:, dd, 128:256], xt, **st)
            # casts on the Scalar engine: its slack is in the QKV phase, and
            # keeping the DVE queue exp-only lets the PE's AV stream run
            # without waiting behind bulk copies.
            nc.scalar.copy(qT[:, t0:t0 + TT], ps_q[:])
            # K^T pre-scaled so scores arrive as z = logit*C1EXP/16
            nc.scalar.mul(kT[:, t0:t0 + TT], ps_k[:], KPRESCALE)
            ps_vt = pj_ps.tile([128, TT], dt.float32, tag="pj")
            for dd in range(n_dblk):
                st = dict(start=(dd == 0), stop=(dd == n_dblk - 1))
                nc.tensor.matmul(ps_vt[:], wqkvT[:, dd, 256:384],
                                 xts[dd], **st)
            vt = vt_pool.tile([128, TT], dt.bfloat16, tag="vt")
            # on Vector: splits the t-tile-boundary eviction burst across
            # engines (Scalar otherwise backs up ~2.4us right here)
            nc.vector.tensor_copy(vt[:], ps_vt[:])

            def finish_v():
                # V^T (e x t) -> V (t x e) via PE transpose, 128x128 blocks.
                # Deferred into the attention loop (filler) so the PE does
                # not stall on the vt cast at the t-tile boundary.
                ps_tv = pj_ps.tile([128, TT], dt.bfloat16, tag="pj")
                for j in range(TT // 128):
                    nc.tensor.transpose(ps_tv[:, j * 128:(j + 1) * 128],
                                        vt[:, j * 128:(j + 1) * 128],
                                        ident[:])
                bb = t0 // s
                for j in range(TT // 128):
                    kb = (t0 % s) // KB + j
                    # both heads in one strided copy (dst [128, 2, 64])
                    nc.scalar.copy(
                        v65[:, bb, :, kb, 0:DH],
                        ps_tv[:, j * 128:(j + 1) * 128].rearrange(
                            "p (h e) -> p h e", h=HEADS_PER_CORE))

            return finish_v

        def attention(bb, qt, fillers=(), final=False):
            """One q-tile of causal attention for both heads of batch bb.
            `fillers`: closures emitting independent PE work (outproj blocks,
            V transposes), paced one per two k-blocks to absorb exp latency
            and keep the PE warm. `final`: this is the kernel's last q-tile —
            pipeline the whole output chain per 128-column diagonal chunk
            instead of leaving a serial outproj tail."""
            fillers = list(fillers)
            nkb_ = (qt + 1) * QT // KB
            f_stride = max(1, (nkb_ - 2) // max(1, len(fillers)))
            tq0 = bb * s + qt * QT
            o65_h0 = o65_ps.tile([DH + 1, QT], dt.float32, tag="o65h0")
            o65_h1 = o65_ps.tile([DH + 1, QT], dt.float32, tag="o65h1")
            nkb = (qt + 1) * QT // KB
            if final:
                fdenb = [ev_pool.tile([1, QT], dt.bfloat16, tag="denb",
                                      name=f"fdenb{h}") for h in range(2)]
                frecw = ev_pool.tile([128, QT], dt.float32, tag="recw",
                                     name="frecw")

            def emit_av(pt, qc0, w, kb):
                st = dict(start=(kb == 0), stop=(kb == nkb - 1))
                nc.tensor.matmul(o65_h0[:, qc0:QT], v65[:, bb, 0, kb, :],
                                 pt[:, 0:w], **st)
                nc.tensor.matmul(o65_h1[:, qc0:QT], v65[:, bb, 1, kb, :],
                                 pt[:, QT:QT + w], **st)
                j = kb - qt * (QT // KB)
                if j >= 0:
                    # after diagonal AV j, ctx columns [j*KB,(j+1)*KB) are
                    # final: evict them now, split across Scalar/Vector.
                    # Chunking kills the 2.7us scalar burst at the tile end
                    # (which stalled the next tile's matmuls and tripped the
                    # HAM clock gate back to 1.2 GHz).
                    c0, c1 = j * KB, (j + 1) * KB
                    nc.scalar.copy(ctxT[0:DH, tq0 + c0:tq0 + c1],
                                   o65_h0[0:DH, c0:c1])
                    nc.vector.tensor_copy(ctxT[DH:2 * DH, tq0 + c0:tq0 + c1],
                                          o65_h1[0:DH, c0:c1])
                    if final:
                        # this 128-col chunk is complete: run its whole
                        # output chain now so no serial tail remains.
                        nc.scalar.copy(fdenb[0][:, c0:c1],
                                       o65_h0[DH:DH + 1, c0:c1])
                        nc.vector.tensor_copy(fdenb[1][:, c0:c1],
                                              o65_h1[DH:DH + 1, c0:c1])
                        fbc = pj_ps.tile([128, KB], dt.float32, tag="pj")
                        nc.tensor.matmul(fbc[0:DH, :], ones1[:, 0:DH],
                                         fdenb[0][:, c0:c1],
                                         tile_position=(0, 0))
                        nc.tensor.matmul(fbc[DH:2 * DH, :], ones1[:, 0:DH],
                                         fdenb[1][:, c0:c1],
                                         tile_position=(0, 64))
                        nc.vector.reciprocal_approx_fast(
                            frecw[:, c0:c1], fbc[:])
                        sl = ctxT[:, tq0 + c0:tq0 + c1]
                        nc.vector.tensor_tensor(sl, sl, frecw[:, c0:c1],
                                                ALU.mult)
                        t0 = tq0 + c0
                        ob = out_sb_pool.tile([128, D], dt.bfloat16,
                                              tag="ob")
                        for e in range(D // 512):
                            ps = pj_ps.tile([128, 512], dt.float32,
                                            tag="pj")
                            nc.tensor.matmul(ps[:], ctxT[:, t0:t0 + 128],
                                             woutT[:, e * 512:(e + 1) * 512])
                            dst = ob[:, e * 512:(e + 1) * 512]
                            if e == 0:
                                nc.scalar.copy(dst, ps[:])
                            else:
                                nc.vector.tensor_copy(dst, ps[:])
                        nc.sync.dma_start(out_d[t0:t0 + 128, :], ob[:])

            pending = []  # AV lagged three k-blocks so exp latency is hidden
            for kb in range(nkb):
                tk0 = bb * s + kb * KB
                j = kb - qt * (QT // KB)  # >= 0 on the diagonal
                qc0 = max(j, 0) * KB      # first valid local q column
                w = QT - qc0
                ps_s = sc_ps.tile([128, 2 * QT], dt.float32, tag="ps_s")
                nc.tensor.matmul(ps_s[:, 0:w], kT[0:64, tk0:tk0 + KB],
                                 qT[0:64, tq0 + qc0:tq0 + QT],
                                 tile_position=(0, 0))
                nc.tensor.matmul(ps_s[:, QT:QT + w], kT[64:128, tk0:tk0 + KB],
                                 qT[64:128, tq0 + qc0:tq0 + QT],
                                 tile_position=(64, 0))
                pt = pt_pool.tile([128, 2 * QT], dt.bfloat16, tag="pt")
                # exp split per head every iteration: Scalar table-exp on one
                # half, DVE EXP16_ANT on the other — halves the latency to
                # pt readiness vs one 1024-wide op, and loads both engines
                # evenly (each ~0.6us per 790ns of PE work). Alternate which
                # engine gets which head so neither systematically gates the
                # first AV matmul of the (lagged) pair.
                sc_half = (kb % 2) * QT
                ve_half = QT - sc_half
                nc.scalar.activation(pt[:, sc_half:sc_half + w],
                                     ps_s[:, sc_half:sc_half + w], AF.Exp,
                                     scale=ACT_SCALE)
                nc.vector._custom_dve(
                    exp16, out=pt[:, ve_half:ve_half + w],
                    in0=ps_s[:, ve_half:ve_half + w],
                    s0=EXP_S0, s1=EXP_S1)
                if j >= 0:
                    for half in (QT, 0):
                        # zero the k>q corner (first KB valid columns)
                        nc.vector.tensor_tensor(
                            pt[:, half:half + KB], pt[:, half:half + KB],
                            tri[:], ALU.mult)
                pending.append((pt, qc0, w, kb))
                if len(pending) > 3:
                    emit_av(*pending.pop(0))
                if fillers and kb >= 2 and (kb - 2) % f_stride == 0:
                    fillers.pop(0)()
            for p in pending:
                emit_av(*p)
            for f in fillers:
                f()
            if final:
                return
            # ctx itself was evicted in chunks along the diagonal; only the
            # denominator rows remain. Evict them DIRECTLY as bf16 — one
            # single-lane op per head instead of the old copy+reciprocal+
            # cast chain ([1,512] ops are lane-serial, ~0.65us EACH on the
            # DVE queue exactly where the next tile's scores wait for it).
            # The reciprocal happens lane-parallel on the broadcast tile in
            # the norm filler instead.
            for h, o65 in ((0, o65_h0), (1, o65_h1)):
                denb = ev_pool.tile([1, QT], dt.bfloat16, tag="denb")
                if h == 0:
                    nc.scalar.copy(denb[:], o65[DH:DH + 1, :])
                else:
                    nc.vector.tensor_copy(denb[:], o65[DH:DH + 1, :])
                rec_saved[(bb, qt, h)] = denb

        def outproj_parts(bb, qt):
            """Output projection for q-tile (bb, qt) as a list of closures
            (norm + 4 t-blocks), to be emitted as attention fillers."""
            tq0 = bb * s + qt * QT

            def norm():
                # partition-broadcast the DENOMINATORS on the PE (ones x den
                # into PSUM, one 64-col tile per head), then one lane-
                # parallel [128,QT] reciprocal (same wall time as a single-
                # lane [1,QT] one) and one DVE multiply normalize ctx^T.
                den0 = rec_saved.pop((bb, qt, 0))
                den1 = rec_saved.pop((bb, qt, 1))
                bc_ps = pj_ps.tile([128, QT], dt.float32, tag="pj")
                nc.tensor.matmul(bc_ps[0:DH, :], ones1[:, 0:DH], den0[:],
                                 tile_position=(0, 0))
                nc.tensor.matmul(bc_ps[DH:2 * DH, :], ones1[:, 0:DH],
                                 den1[:], tile_position=(0, 64))
                recw = ev_pool.tile([128, QT], dt.float32, tag="recw")
                nc.vector.reciprocal_approx_fast(recw[:], bc_ps[:])
                sl = ctxT[:, tq0:tq0 + QT]
                nc.vector.tensor_tensor(sl, sl, recw[:], ALU.mult)

            def tb(tb4):
                t0 = bb * s + qt * QT + tb4 * 128
                ob = out_sb_pool.tile([128, D], dt.bfloat16, tag="ob")
                for e in range(D // 512):
                    ps = pj_ps.tile([128, 512], dt.float32, tag="pj")
                    nc.tensor.matmul(ps[:], ctxT[:, t0:t0 + 128],
                                     woutT[:, e * 512:(e + 1) * 512])
                    dst = ob[:, e * 512:(e + 1) * 512]
                    if (2 * tb4 + e) % 2 == 0:
                        nc.scalar.copy(dst, ps[:])
                    else:
                        nc.vector.tensor_copy(dst, ps[:])
                nc.sync.dma_start(out_d[t0:t0 + 128, :], ob[:])

            return [norm] + [
                (lambda tb4=tb4: tb(tb4)) for tb4 in range(QT // 128)]

        # Emission: QKV t-tiles interleaved with the attention q-tiles they
        # unblock. The output projection of the PREVIOUS q-tile and the V
        # transposes are spread as fillers inside the attention k-loop, so
        # the in-order PE always has independent work while the exp engines
        # catch up, and the kernel tail shrinks to one q-tile's outproj.
        tiles_per_batch = n_ttiles // b  # == n_qt
        last = tiles_per_batch - 1
        # startup: narrow (1-tile) prefetches, interleaved across batches,
        # so qkv(batch1, t-tile 0) isn't queued behind ~11us of batch-0 DMA
        # (which left the PE idle until ~23us and tripped the HAM gate).
        for k in range(2):
            for bb in range(b):
                prefetch_xt(bb * tiles_per_batch + k, n_tt=1)
        for bb in range(b):
            prefetch_xt(bb * tiles_per_batch + 2, n_tt=2)
        for tt in range(tiles_per_batch):
            qkv_fin = [qkv_ttile(bb * tiles_per_batch + tt)
                       for bb in range(b)]
            for bb in range(b):
                fill = []
                if tt == 0:
                    # q-tile 0's first AV consumes this tile's fresh V
                    # blocks almost immediately: emit the transposes now
                    # (a filler slot would deadlock the in-order PE queue)
                    qkv_fin[bb]()
                else:
                    fill.append(qkv_fin[bb])
                    fill += outproj_parts(bb, tt - 1)
                fin = tt == last and bb == b - 1
                if fin:
                    fill += outproj_parts(0, tt)
                attention(bb, tt, fill, final=fin)
            # prefetch AFTER the out-stores: the outs then sit at the SP FIFO
            # head (ob buffers recycle promptly) while the 2-iteration lead
            # still lands x^T well before its QKV consumer
            if tt % 2 == 1 and tt + 3 < tiles_per_batch:
                for bb in range(b):
                    prefetch_xt(bb * tiles_per_batch + tt + 3, n_tt=2)

    return nc


def _get_kernel(b, s):
    key = (b, s)
    if key not in _cache:
        from concourse import bacc
        nc = bacc.Bacc()
        _build(nc, b, s)
        nc.finalize()
        _cache[key] = nc
    return _cache[key]


def _prep_inputs(x, Wqkv, Wout):
    """Host-side shard + transpose + bf16 cast. Returns list of in_maps."""
    b, s, d = x.shape
    xT = np.ascontiguousarray(
        x.reshape(b * s, d).astype(ml_dtypes.bfloat16).T)  # (d, b*s)
    n_dblk = d // 128
    in_maps = []
    for i in range(N_CORES):
        r0 = i * 128
        wq = Wqkv[r0:r0 + 128]            # (128, d)
        wk = Wqkv[d + r0:d + r0 + 128]
        wv = Wqkv[2 * d + r0:2 * d + r0 + 128]
        wT = np.concatenate([wq.T, wk.T, wv.T], axis=1)  # (d, 384)
        wT = wT.reshape(n_dblk, 128, 3 * 128).astype(ml_dtypes.bfloat16)
        woT = Wout[:, r0:r0 + 128].T.astype(ml_dtypes.bfloat16)
        woT = np.ascontiguousarray(woT)
        in_maps.append({"xT": xT, "wqkvT": wT, "woutT": woT})
    return in_maps


_runner_cache = {}


def _make_runner(nc, n_cores):
    """Like bass2jax.run_bass_via_pjrt but with the jitted executable built
    once and cached, and output zero-buffers created on-device."""
    import jax
    from jax.sharding import Mesh, PartitionSpec
    from jax.experimental.shard_map import shard_map
    import concourse.mybir as mybir
    from concourse import bass2jax

    bass2jax.install_neuronx_cc_hook()
    partition_name = (nc.partition_id_tensor.name
                      if nc.partition_id_tensor else None)
    in_names, out_names, out_avals = [], [], []
    for alloc in nc.m.functions[0].allocations:
        if not isinstance(alloc, mybir.MemoryLocationSet):
            continue
        name = alloc.memorylocations[0].name
        if alloc.kind == "ExternalInput":
            if name != partition_name:
                in_names.append(name)
        elif alloc.kind == "ExternalOutput":
            out_names.append(name)
            out_avals.append(jax.core.ShapedArray(
                tuple(alloc.tensor_shape), mybir.dt.np(alloc.dtype)))
    n_params = len(in_names)
    n_outs = len(out_names)
    bind_names = list(in_names) + list(out_names)
    if partition_name is not None:
        bind_names.append(partition_name)

    def _body(*args):
        operands = list(args)
        if partition_name is not None:
            operands.append(bass2jax.partition_id_tensor())
        outs = bass2jax._bass_exec_p.bind(
            *operands,
            out_avals=tuple(out_avals),
            in_names=tuple(bind_names),
            out_names=tuple(out_names),
            lowering_input_output_aliases=(),
            sim_require_finite=True,
            sim_require_nnan=True,
            nc=nc,
        )
        return tuple(outs)

    devices = jax.devices()[:n_cores]
    mesh = Mesh(np.array(devices), ("core",))
    sharded = jax.jit(
        shard_map(
            _body, mesh=mesh,
            in_specs=(PartitionSpec("core"),) * (n_params + n_outs),
            out_specs=(PartitionSpec("core"),) * n_outs,
            check_rep=False),
        donate_argnums=tuple(range(n_params, n_params + n_outs)),
        keep_unused=True)

    def run(in_maps):
        concat_in = [
            np.concatenate([np.asarray(m[name]) for m in in_maps], axis=0)
            for name in in_names]
        concat_zeros = [
            np.zeros((n_cores * a.shape[0], *a.shape[1:]), a.dtype)
            for a in out_avals]
        out_arrs = sharded(*concat_in, *concat_zeros)
        return [
            {name: np.asarray(out_arrs[i]).reshape(
                n_cores, *out_avals[i].shape)[c]
             for i, name in enumerate(out_names)}
            for c in range(n_cores)]

    return run


def kernel(x, Wqkv, Wout, _trace=False):
    b, s, d = x.shape
    nc = _get_kernel(b, s)
    in_maps = _prep_inputs(np.asarray(x), np.asarray(Wqkv), np.asarray(Wout))
    if _trace:
        from concourse.bass_utils import run_bass_kernel_spmd
        res = run_bass_kernel_spmd(nc, in_maps,
                                   core_ids=list(range(N_CORES)), trace=True)
        results = res.results
        kernel.last_results = res
    else:
        key = id(nc)
        if key not in _runner_cache:
            _runner_cache[key] = _make_runner(nc, N_CORES)
        results = _runner_cache[key](in_maps)
    acc = results[0]["partial_out"].astype(np.float32)
    for i in range(1, N_CORES):
        acc = acc + results[i]["partial_out"]
    return acc.reshape(b, s, d)

